# revision 38
# baseline (speedup 1.0000x reference)
"""Trainium2 Bass kernel for nn_MultiHeadAttention_72378788872456.

Sharding (8 cores): core c handles batch b = c//4 and head group g = c%4
(heads 4g..4g+3).  Tensor-parallel on heads within each batch's 4-core
group; partial outputs are summed on the host (no device collective).

Layouts (all "transposed" so no device-side transposes are needed):
  qT/kT/vT inputs: [chunk, 128=d-tile, DT, 512] fp16 (contraction d on
  partitions, chunk-major for contiguous DMA)
  sin/cos rope maps precomputed on host: [128, n] fp16
  q/k after proj+rope: per head-pair tiles [128 = 2*64 k-dims, n] fp16
  scores S^T: [m-tile, n] PSUM tiles; exp on ACT -> f32r
  o^T accum: [128 = 2*64 v-dims, n] per pair (col-tiled matmuls);
  softmax denominators accumulate in a separate PSUM bank at partitions
  {0,32,64,96} via 4-way col-tiled ones-stationary matmuls.
  normalization: reciprocal_approx_fast + ebc broadcast matmul.
  output projection emits natural [n, d] f32 partials.
"""

import math
import numpy as np

# ---------------------------------------------------------------- constants
B, N, M, D, H, K, V = 2, 2048, 2048, 1024, 16, 64, 64
MAX_WAVELENGTH = 10000.0
SCALE_FACTOR = 1.0
N_CORES = 8
GROUP = 4           # cores per batch (tensor-parallel group)
HLOC = 4            # heads per core
PAIRS = HLOC // 2   # head-pairs per core
P = 128
FREE = 512          # matmul moving free-dim / n-chunk granularity

_COMPILED = {}


def build_nc(n=N, m=M, d=D, n_cores=N_CORES, shared_maps=True):
    """Build the SPMD Bass program (identical on every core)."""
    import concourse.bass as bass
    import concourse.mybir as mybir
    import concourse.tile as tile
    from concourse import bacc

    dt = mybir.dt
    f32 = dt.float32
    f32r = dt.float32r
    f16 = dt.float16
    AF = mybir.ActivationFunctionType
    ALU = mybir.AluOpType

    DT = d // P           # d tiles (contraction steps) for projections
    NC4 = n // FREE       # n chunks
    MT = m // P           # m tiles
    MC4 = m // FREE       # m chunks
    NTPC = FREE // P      # n tiles per chunk (outproj stationaries)
    DC = d // FREE        # d chunks in outproj output

    nc = bacc.Bacc("TRN2", target_bir_lowering=False, debug=False,
                   num_devices=n_cores)

    # ------------------------------------------------ DRAM I/O declarations
    qT_d = nc.dram_tensor("qT", [NC4, P, DT, FREE], f16, kind="ExternalInput").ap()
    kT_d = nc.dram_tensor("kT", [MC4, P, DT, FREE], f16, kind="ExternalInput").ap()
    vT_d = nc.dram_tensor("vT", [MC4, P, DT, FREE], f16, kind="ExternalInput").ap()
    pq_d = nc.dram_tensor("pq", [P, DT, 2 * P], f16, kind="ExternalInput").ap()
    pk_d = nc.dram_tensor("pk", [P, DT, 2 * P], f16, kind="ExternalInput").ap()
    pv_d = nc.dram_tensor("pv", [P, DT, 2 * P], f16, kind="ExternalInput").ap()
    po_d = nc.dram_tensor("po", [P, PAIRS, d], f16, kind="ExternalInput").ap()
    ksin_d = nc.dram_tensor("ksin", [P, m], f16, kind="ExternalInput").ap()
    kcos_d = nc.dram_tensor("kcos", [P, m], f16, kind="ExternalInput").ap()
    if not shared_maps:
        qsin_d = nc.dram_tensor("qsin", [P, n], f16, kind="ExternalInput").ap()
        qcos_d = nc.dram_tensor("qcos", [P, n], f16, kind="ExternalInput").ap()
    ebc_d = nc.dram_tensor("ebc", [P, P], f32r, kind="ExternalInput").ap()
    vones_d = nc.dram_tensor("vones", [P, m // P, HLOC], f32,
                             kind="ExternalInput").ap()
    bf16 = dt.bfloat16
    out_d = nc.dram_tensor("out_part", [NC4, FREE, d], bf16,
                           kind="ExternalOutput").ap()

    SWAP_MASK = [i ^ 1 for i in range(32)]

    from collections import deque

    with tile.TileContext(nc) as tc:
        with (
            tc.tile_pool(name="persist", bufs=1) as persist,
            tc.tile_pool(name="instream", bufs=10) as instream,
            tc.tile_pool(name="mtmp", bufs=4) as mtmp,
            tc.tile_pool(name="expp", bufs=4) as expp,
            tc.tile_pool(name="nrm", bufs=2) as nrm,
            tc.tile_pool(name="otn", bufs=4) as otnp,
            tc.tile_pool(name="stg", bufs=3) as stgp,
            # PSUM: stps 2x[128,1024]=4 banks, otps 2x[128,512]=2 banks,
            # mmps 2x[128,512]=2 banks -> 8 banks exactly.
            tc.tile_pool(name="stps", bufs=2, space="PSUM") as stps,
            tc.tile_pool(name="otps", bufs=2, space="PSUM") as otps,
            tc.tile_pool(name="mmps", bufs=2, space="PSUM") as mmps,
        ):
            # ---------------------------------------------------- constants
            po_sb = persist.tile([P, PAIRS, d], f16, tag="po")
            ksin_sb = persist.tile([P, m], f16, tag="ksin")
            kcos_sb = persist.tile([P, m], f16, tag="kcos")
            if shared_maps:
                qsin_sb, qcos_sb = ksin_sb, kcos_sb
            else:
                qsin_sb = persist.tile([P, n], f16, tag="qsin")
                qcos_sb = persist.tile([P, n], f16, tag="qcos")
            rrf_sb = persist.tile([P, 2, FREE], f32r, tag="rrf")
            dcp_sb = persist.tile([P, 2, FREE], f32, tag="dcp")
            ebc_sb = persist.tile([P, P], f32r, tag="ebc")
            vstage = persist.tile([P, MT, HLOC], f32, tag="vstage")
            vsb = persist.tile([P, MT, HLOC * 65], f32r, tag="vsb")

            # weights + input streams on the sync queue (consumption order);
            # maps/po/ebc/vones staging on the scalar queue.
            pq_sb = persist.tile([P, DT, 2 * P], f16, tag="pq")
            pk_sb = persist.tile([P, DT, 2 * P], f16, tag="pk")
            pv_sb = persist.tile([P, DT, 2 * P], f16, tag="pv")
            ktin = [instream.tile([P, DT, FREE], f16, tag="instream",
                                  name=f"tk{c}") for c in range(MC4)]
            vtin = [instream.tile([P, DT, FREE], f16, tag="instream",
                                  name=f"tv{c}") for c in range(MC4)]
            qtin = [instream.tile([P, DT, FREE], f16, tag="instream",
                                  name=f"tq{c}") for c in range(NC4)]
            # DMA issue order == first-consumer order; sync queue carries the
            # critical path, scalar queue the rest.
            nc.sync.dma_start(pk_sb[:], pk_d[:, :, :])
            nc.sync.dma_start(ktin[0][:], kT_d[0, :, :, :])
            nc.sync.dma_start(pq_sb[:], pq_d[:, :, :])
            nc.sync.dma_start(qtin[0][:], qT_d[0, :, :, :])
            nc.scalar.dma_start(ksin_sb[:], ksin_d[:, :])
            nc.scalar.dma_start(kcos_sb[:], kcos_d[:, :])
            nc.sync.dma_start(pv_sb[:], pv_d[:, :, :])
            nc.sync.dma_start(vtin[0][:], vT_d[0, :, :, :])
            nc.sync.dma_start(ktin[1][:], kT_d[1, :, :, :])
            nc.sync.dma_start(vtin[1][:], vT_d[1, :, :, :])
            if not shared_maps:
                nc.scalar.dma_start(qsin_sb[:], qsin_d[:, :])
                nc.scalar.dma_start(qcos_sb[:], qcos_d[:, :])
            nc.scalar.dma_start(vstage[:], vones_d[:, :, :])
            nc.sync.dma_start(ktin[2][:], kT_d[2, :, :, :])
            nc.sync.dma_start(vtin[2][:], vT_d[2, :, :, :])
            nc.sync.dma_start(ktin[3][:], kT_d[3, :, :, :])
            nc.sync.dma_start(vtin[3][:], vT_d[3, :, :, :])
            nc.scalar.dma_start(po_sb[:], po_d[:, :, :])
            nc.scalar.dma_start(ebc_sb[:], ebc_d[:, :])
            for c in range(1, NC4):
                nc.sync.dma_start(qtin[c][:], qT_d[c, :, :, :])
            nc.vector.memset(dcp_sb[:], 1.0)
            # ones column of the V stationaries (f32 -> f32r cast on DVE;
            # a direct strided DMA would be thousands of 4B descriptors)
            nc.vector.tensor_copy(
                vsb[:].rearrange("p m (h w) -> p m h w", h=HLOC)[:, :, :, 64],
                vstage[:])

            krope = [[None] * MC4 for _ in range(PAIRS)]
            qrope = [[None] * NC4 for _ in range(PAIRS)]

            def project_rope_chunk(tin, p_sb, sinm, cosm, pair, name, pool,
                                   ptag="st"):
                """One [P, FREE] rope'd projection tile for one pair (fp16)."""
                ps = pool.tile([P, FREE], f32, tag=ptag, name=f"psp_{name}")
                for t in range(DT):
                    nc.tensor.matmul(
                        ps[:], p_sb[:, t, pair * P:(pair + 1) * P],
                        tin[:, t, :],
                        start=(t == 0), stop=(t == DT - 1))
                # rope (interleaved k-dims): out = x*cos + swap(x)*sin'
                t1 = mtmp.tile([P, FREE], f32, tag="t1")
                nc.vector.tensor_tensor(t1[:], ps[:], cosm, ALU.mult)
                xsw = mtmp.tile([P, FREE], f32, tag="xsw")
                nc.vector.stream_shuffle(xsw[:], ps[:], SWAP_MASK)
                u = mtmp.tile([P, FREE], f32, tag="u")
                nc.gpsimd.tensor_tensor(u[:], xsw[:], sinm, ALU.mult)
                out = persist.tile([P, FREE], f16, tag=f"rope_{name}",
                                   name=f"r_{name}")
                nc.gpsimd.tensor_tensor(out[:], t1[:], u[:], ALU.add)
                return out

            def emit_kproj(c, pr, pool, ptag="st"):
                krope[pr][c] = project_rope_chunk(
                    ktin[c], pk_sb, ksin_sb[:, c * FREE:(c + 1) * FREE],
                    kcos_sb[:, c * FREE:(c + 1) * FREE],
                    pr, f"k{pr}_{c}", pool, ptag)

            def emit_vproj(c, mi4, pool, ptag="st"):
                mi = c * (FREE // P) + mi4
                ps = pool.tile([P, FREE], f32, tag=ptag, name=f"psv_{mi}")
                for t in range(DT):
                    nc.tensor.matmul(
                        ps[:, 0:2 * P], vtin[c][:, t, mi4 * P:(mi4 + 1) * P],
                        pv_sb[:, t, :],
                        start=(t == 0), stop=(t == DT - 1))
                nc.vector.tensor_copy(
                    vsb[:, mi, :].rearrange("p (h w) -> p h w", h=HLOC)[:, :, 0:64],
                    ps[:, 0:2 * P].rearrange("p (h w) -> p h w", h=HLOC))

            def emit_qproj(c, pr, pool, ptag="st"):
                qrope[pr][c] = project_rope_chunk(
                    qtin[c], pq_sb, qsin_sb[:, c * FREE:(c + 1) * FREE],
                    qcos_sb[:, c * FREE:(c + 1) * FREE],
                    pr, f"q{pr}_{c}", pool, ptag)

            def emit_st(c, pr, mi):
                """S^T tile for (chunk c, pair pr, m-tile mi): [128, 1024]."""
                stp = stps.tile([P, 2 * FREE], f32, tag="st",
                                name=f"st_{c}_{pr}_{mi}")
                kc = krope[pr][mi // (FREE // P)]
                msl = slice((mi % (FREE // P)) * P, (mi % (FREE // P) + 1) * P)
                for h in range(2):
                    hp = h * 64
                    nc.tensor.matmul(
                        stp[:, h * FREE:(h + 1) * FREE],
                        kc[hp:hp + 64, msl],
                        qrope[pr][c][hp:hp + 64, :],
                        start=True, stop=True,
                        tile_position=(hp, 0))
                return stp

            def emit_outproj(cc, otn_tiles):
                """[FREE, d] f32 partial for chunk cc -> DRAM."""
                for nt in range(NTPC):
                    for dc in range(DC):
                        ops_ = mmps.tile([P, FREE], f32, tag="mm")
                        for t in range(PAIRS):
                            nc.tensor.matmul(
                                ops_[:], otn_tiles[t][:, nt * P:(nt + 1) * P],
                                po_sb[:, t, dc * FREE:(dc + 1) * FREE],
                                start=(t == 0), stop=(t == PAIRS - 1))
                        stg = stgp.tile([P, FREE], bf16, tag="stg")
                        with nc.allow_low_precision(reason="bf16 partials"):
                            nc.vector.tensor_copy(stg[:], ops_[:])
                        nc.sync.dma_start(
                            out_d[cc, nt * P:(nt + 1) * P,
                                  dc * FREE:(dc + 1) * FREE],
                            stg[:])

            # ------------------------------------------------ prologue
            # the bare minimum before attention: k and q projections for
            # chunk 0 (everything else drips into the attention loop)
            for pr in range(PAIRS):
                emit_kproj(0, pr, stps)
            for pr in range(PAIRS):
                emit_qproj(0, pr, stps)

            # per-mi emission schedule for the remaining projections inside
            # (c0, pr0): vproj(vsb[mi']) must be emitted before O(mi'), and
            # kproj(c') before the S^T prefetch that reads krope[c'].
            sched00 = {mi: [] for mi in range(MT)}
            for cc in range(MC4):
                for j in range(FREE // P):
                    sched00[4 * cc + j].append(
                        lambda cc=cc, j=j: emit_vproj(cc, j, mmps, "mm"))
            for cc in (1, 2, 3):
                for pr in range(PAIRS):
                    sched00[4 * (cc - 1) + pr].append(
                        lambda cc=cc, pr=pr: emit_kproj(cc, pr, mmps, "mm"))
            # q projections for chunks 1-3 drain during (c0, pr1)
            backfill = deque()
            for cc in (1, 2, 3):
                for pr in range(PAIRS):
                    backfill.append(
                        lambda cc=cc, pr=pr: emit_qproj(cc, pr, mmps, "mm"))

            # ------------------------------------------------ main loop
            pending_outproj = None     # (chunk, otn tiles)
            pending_norm = None        # deferred normB closure
            pending_recip = None       # deferred reciprocal closure
            otn_by_chunk = {cc: [None, None] for cc in range(NC4)}
            stp_next = None            # pipelined S^T tile

            def make_normB(cc, pr, oraw, dslot):
                """PE broadcast matmul + final multiply; emitted ~8 mi after
                the reciprocal so the PE queue never waits on it."""
                def emit():
                    rb = mmps.tile([P, FREE], f32, tag="mm",
                                   name=f"rb_{cc}_{pr}")
                    nc.tensor.matmul(rb[:], ebc_sb[0:33, :],
                                     rrf_sb[0:33, dslot, :],
                                     start=True, stop=True)
                    rbs = nrm.tile([P, FREE], f32, tag="rbs",
                                   name=f"rbs_{cc}_{pr}")
                    nc.vector.tensor_copy(rbs[:], rb[:])
                    ot = otnp.tile([P, FREE], f16, tag="otn",
                                   name=f"ot_{cc}_{pr}")
                    nc.gpsimd.tensor_tensor(ot[:], oraw[:], rbs[:], ALU.mult)
                    otn_by_chunk[cc][pr] = ot
                return emit

            for c in range(NC4):
                for pr in range(PAIRS):
                    # o^T accumulators: [65, FREE] per head (64 v-dims +
                    # softmax denominator from the vsb ones column)
                    pot = [otps.tile([65, FREE], f32, tag="ot",
                                     name=f"pot_{c}_{pr}_{h}")
                           for h in range(2)]
                    if stp_next is None:
                        stp_next = emit_st(c, pr, 0)
                    stp_cur = stp_next
                    for mi in range(MT):
                        # drip-feed deferred PE work into the ACT-bound loop
                        if c == 0 and pr == 0:
                            for fn in sched00[mi]:
                                fn()
                        elif backfill and mi % 2 == 0:
                            backfill.popleft()()
                        if mi == 4 and pending_recip is not None:
                            pending_recip()
                            pending_recip = None
                        if mi == 10 and pending_norm is not None:
                            pending_norm()
                            pending_norm = None
                        if pr == 1 and mi == 2 and pending_outproj is not None:
                            emit_outproj(*pending_outproj)
                            pending_outproj = None
                        # prefetch next S^T (next mi / next pair / next chunk)
                        if mi + 1 < MT:
                            stp_next = emit_st(c, pr, mi + 1)
                        elif pr + 1 < PAIRS:
                            stp_next = emit_st(c, pr + 1, 0)
                        elif c + 1 < NC4:
                            stp_next = emit_st(c + 1, 0, 0)
                        else:
                            stp_next = None
                        ex = expp.tile([P, 2 * FREE], f32r, tag="exp",
                                       name=f"ex_{c}_{pr}_{mi}")
                        nc.scalar.activation(ex[:], stp_cur[:], AF.Exp)
                        for h in range(2):
                            hc = (2 * pr + h) * 65
                            nc.tensor.matmul(
                                pot[h][:], vsb[:, mi, hc:hc + 65],
                                ex[:, h * FREE:(h + 1) * FREE],
                                start=(mi == 0), stop=(mi == MT - 1))
                        stp_cur = stp_next

                    # ------------- normalization part A for (c, pr):
                    # evacuate PSUM immediately (denom rows + raw o) so the
                    # accumulators free for the next pair; the reciprocal
                    # runs on DVE concurrently with the next pair's stream.
                    # rrf has two slots (ping-pong by pair parity) so the
                    # deferred normB reads stable data.
                    dslot = pr
                    for h in range(2):
                        nc.vector.tensor_copy(
                            dcp_sb[32 * h:32 * h + 1, dslot, :],
                            pot[h][64:65, :])
                    oraw = nrm.tile([P, FREE], f32, tag="oraw",
                                    name=f"oraw_{c}_{pr}")
                    nc.scalar.copy(oraw[0:64, :], pot[0][0:64, :])
                    nc.scalar.copy(oraw[64:128, :], pot[1][0:64, :])
                    def make_recip(dslot):
                        def emit():
                            with nc.allow_low_precision(
                                    reason="denom recip to f32r"):
                                nc.vector.reciprocal(
                                    rrf_sb[0:33, dslot, :],
                                    dcp_sb[0:33, dslot, :])
                        return emit
                    pending_recip = make_recip(dslot)
                    pending_norm = make_normB(c, pr, oraw, dslot)

                if c + 1 == NC4:
                    # last chunk: flush immediately
                    pending_recip()
                    pending_recip = None
                    pending_norm()
                    pending_norm = None
                pending_outproj = (c, otn_by_chunk[c])

            emit_outproj(*pending_outproj)

    nc.compile()
    return nc


# ------------------------------------------------------------------- host

def _rope_maps(positions, length):
    """Host-precomputed rope sin/cos maps [128, length] fp16 (interleaved
    k-dim layout: row p holds original k index (p%2)*32 + p//2)."""
    jj = np.arange(P) % 64
    j_idx = jj // 2
    half = jj % 2
    frac = 2.0 * j_idx / 64.0
    invt = (MAX_WAVELENGTH ** (-frac)) / SCALE_FACTOR        # [128]
    phase = positions.astype(np.float64)[None, :] * invt[:, None]
    sign = np.where(half == 0, -1.0, 1.0)
    sinm = (np.sin(phase) * sign[:, None]).astype(np.float16)
    cosm = np.cos(phase).astype(np.float16)
    return sinm, cosm


def _prep_core_inputs(query, q_positions, key, k_positions, value,
                      P_q, P_k, P_v, P_o, core, n=N, m=M, d=D,
                      shared_maps=True):
    """Build the per-core input map (numpy, host-side shard/layout prep)."""
    b = core // GROUP
    g = core % GROUP
    DT = d // P
    hsl = slice(g * HLOC, (g + 1) * HLOC)

    def t_in(x, length):  # [length, d] -> [length//FREE, P, DT, FREE] fp16
        a = x.T.reshape(DT, P, length).transpose(1, 0, 2)       # [P, DT, len]
        a = a.reshape(P, DT, length // FREE, FREE).transpose(2, 0, 1, 3)
        return np.ascontiguousarray(a.astype(np.float16))

    # interleaved k-dim order: stationary col c (per head) holds original
    # k index (c%2)*32 + c//2, so the rope partner sits on the adjacent
    # partition (stream_shuffle-able swap).
    KPERM = np.array([(c % 2) * 32 + c // 2 for c in range(64)])

    def pack_pqk(Pm):  # [HLOC, d, 64] -> [P, DT, 2*P] head-pair stationaries
        out = np.empty((P, DT, 2 * P), np.float16)
        for p in range(PAIRS):
            for hl in range(2):
                h = 2 * p + hl
                out[:, :, p * P + hl * 64: p * P + hl * 64 + 64] = \
                    Pm[h].reshape(DT, P, 64).transpose(1, 0, 2)[:, :, KPERM]
        return np.ascontiguousarray(out)

    def pack_pv(Pm):  # [HLOC, d, 64] -> [P, DT, 256] (hv on free)
        return np.ascontiguousarray(
            Pm.reshape(HLOC, DT, P, 64).transpose(2, 1, 0, 3)
            .reshape(P, DT, 2 * P).astype(np.float16))

    def pack_po(Pm):  # [HLOC, d, V] -> [P, PAIRS, d];  hv = t*128 + p
        out = np.empty((P, PAIRS, d), np.float16)
        for t in range(PAIRS):
            for hl in range(2):
                h = 2 * t + hl
                out[hl * 64:(hl + 1) * 64, t, :] = Pm[h].T  # [V, d]
        return np.ascontiguousarray(out)

    # broadcast stationary: output rows 0-63 copy reciprocal row 0,
    # rows 64-127 copy row 1
    ebc = np.zeros((P, P), np.float32)
    ebc[0, 0:64] = 1.0
    ebc[32, 64:128] = 1.0

    ksin, kcos = _rope_maps(np.asarray(k_positions[b]), m)

    out_map = {
        "qT": t_in(query[b], n),
        "kT": t_in(key[b], m),
        "vT": t_in(value[b], m),
        "pq": pack_pqk(P_q[hsl].astype(np.float32)),
        "pk": pack_pqk(P_k[hsl].astype(np.float32)),
        "pv": pack_pv(P_v[hsl]),
        "po": pack_po(P_o[hsl]),
        "ksin": ksin, "kcos": kcos,
        "ebc": ebc,
        "vones": np.ones((P, m // P, HLOC), np.float32),
    }
    if not shared_maps:
        qsin, qcos = _rope_maps(np.asarray(q_positions[b]), n)
        out_map["qsin"] = qsin
        out_map["qcos"] = qcos
    return out_map


def assemble_output(results, n=N, d=D):
    """Sum per-core [NC4, FREE, d] partials into the full [B, n, d]."""
    out = np.zeros((B, n, d), np.float32)
    for core in range(N_CORES):
        b = core // GROUP
        part = np.asarray(results[core]["out_part"]).reshape(n, d)
        out[b] += part
    return out


def kernel(query, q_positions, key, k_positions, value, mask=None,
           P_q=None, P_k=None, P_v=None, P_o=None, **_unused):
    from concourse.bass_utils import run_bass_kernel_spmd

    query = np.asarray(query, np.float32)
    key = np.asarray(key, np.float32)
    value = np.asarray(value, np.float32)
    q_positions = np.asarray(q_positions, np.int32)
    k_positions = np.asarray(k_positions, np.int32)
    P_q = np.asarray(P_q, np.float32)
    P_k = np.asarray(P_k, np.float32)
    P_v = np.asarray(P_v, np.float32)
    P_o = np.asarray(P_o, np.float32)

    shared = (N == M) and np.array_equal(q_positions, k_positions)
    key_dims = (N, M, D, shared)
    if key_dims not in _COMPILED:
        _COMPILED[key_dims] = build_nc(N, M, D, shared_maps=shared)
    nc = _COMPILED[key_dims]

    in_maps = [
        _prep_core_inputs(query, q_positions, key, k_positions, value,
                          P_q, P_k, P_v, P_o, core, shared_maps=shared)
        for core in range(N_CORES)
    ]
    res = run_bass_kernel_spmd(nc, in_maps, list(range(N_CORES)))
    return assemble_output(res.results)


if __name__ == "__main__":
    print("building...")
    build_nc()
    print("ok")
